# revision 74
# baseline (speedup 1.0000x reference)
"""AttnBlock (GroupNorm -> QKV 1x1 -> full NxN attention -> proj -> residual)
for Trainium2, SPMD over 8 NeuronCores.

Sharding: data-parallel over batch (2) x query-pixel blocks (4 of 1024 px).
Each core receives its batch image x [C, N] PERMUTED so that its own query
block occupies pixels [0, NQ); attention is permutation-invariant over keys.
No collectives.

v3 structure — K and V are never materialized.  Weight products fold on the
host; GroupNorm folds into tiny per-channel vectors on device:

  hn = A*x + B (per-channel).  With P0 = s*Wk^T@Wq and W2 = Wp@Wv (host):
    S[k,q]  = x[:,k]^T @ QtA[:,q]   (+ per-q consts that cancel in softmax)
    QtA     = diag(A) (P0^T diag(A) x_q + P0^T B + s Wk^T bq)
    out     = W2 (A . Z0) / den + (W2 B + Wp bv + bp) + x,   Z0 = x @ P^T

  Device tensors: x bf16 (stats + residual), x8 fp8 [c,n] (S lhsT + query
  rhs), xt8 fp8 [k,c] pair-interleaved (Z lhsT; host-transposed), p0t bf16,
  w2t bf16, one packed const vector.  GroupNorm A enters via a [C,C] lhsT
  row-scale (P0A8 fold), a drain scale on Qt, and a drain scale on Z0; all
  B / bias terms ride along as drain biases or the proj bias.

  Heavy matmuls (Qt production, S, Z0, softmax denominators) run fp8 e4m3
  DoubleRow (0.5 cyc/row); proj runs bf16.

  Scheduling notes (cost-model driven):
  - All input DMAs ride the SP queue in priority order (p0t, x, x8, xt8,
    w2t); the DMA engines are a single ~25us serial resource, so order is
    everything.  Stats windows pipeline with the x chunks as they land.
  - Per 512-px window: DVE does the sums (tensor_scalar+accum runs 4x on
    bf16) + one sumsq; ACT two sumsq (Square+accum); Pool one sumsq.
  - ACT loads the Sqrt table set first (it also holds Identity/Square for
    the stats window), switches to the Exp set once, then owns the exp
    stream; every other drain lives on DVE/Pool.
  - S->exp->Z0 pipeline: S pairs on PE feed ACT exp; Z0/den DoubleRow
    matmuls trail one pair behind; the previous qpass's proj (and the
    bp_dev matvec) slot into PE gaps of the exp-bound stream.
"""

from contextlib import ExitStack

import numpy as np

import concourse.bacc as bacc
import concourse.bass as bass
import concourse.mybir as mybir
import concourse.tile as tile

F32 = mybir.dt.float32
F32R = mybir.dt.float32r
BF16 = mybir.dt.bfloat16
FP8 = mybir.dt.float8e4
AF = mybir.ActivationFunctionType
MUL = mybir.AluOpType.mult
ADD = mybir.AluOpType.add

SC_P0 = 64.0      # fp8 P0A lhsT pre-scale (dodges e4m3 subnormals)
SC_QT = 16.0      # fp8 Qt storage scale


def build_program(C=512, G=32, N=4096, NQ=1024, eps=1e-5, precision="tf32"):
    """Emit the per-core Bass program (SPMD; per-core data differs only)."""
    P = 128
    CS = C // P                  # channel subtiles
    KT = N // P                  # key/pixel tiles
    NCH = 512                    # x DMA chunk / stats window (px)
    NCHUNKS = N // NCH
    QP = min(512, NQ)            # query-pass width
    QPASSES = NQ // QP
    cpg = C // G                 # channels per group
    GPS = P // cpg               # groups per channel-subtile
    assert C % P == 0 and N % P == 0 and NQ % QP == 0 and P % cpg == 0
    SDT = BF16

    nc = bacc.Bacc(None, target_bir_lowering=False)

    NX = N - 1024        # bf16 x ships only for stats windows + residual;
    x_d = nc.dram_tensor("x", [C, NX], SDT, kind="ExternalInput")
    x8_d = nc.dram_tensor("x8", [C, N], FP8, kind="ExternalInput")
    xt8_d = nc.dram_tensor("xt8", [P, KT // 2, 2, C], FP8, kind="ExternalInput")
    p0t_d = nc.dram_tensor("p0t", [C, C], SDT, kind="ExternalInput")
    w2t_d = nc.dram_tensor("w2t", [C, C], SDT, kind="ExternalInput")
    CPW = 4 * CS + GPS + P
    cpk_d = nc.dram_tensor("cpk", [P, CPW], F32, kind="ExternalInput")
    out_d = nc.dram_tensor("out", [C, NQ], SDT, kind="ExternalOutput")

    x_r = x_d[:, :].rearrange("(s p) n -> p s n", p=P)
    x8_r = x8_d[:, :].rearrange("(s p) n -> p s n", p=P)
    p0_r = p0t_d[:, :].rearrange("(s p) o -> p s o", p=P)
    w2_r = w2t_d[:, :].rearrange("(s p) o -> p s o", p=P)
    out_r = out_d[:, :].rearrange("(s p) n -> p s n", p=P)

    with tile.TileContext(nc) as tc, ExitStack() as st:
        const = st.enter_context(tc.tile_pool(name="const", bufs=1))
        big = st.enter_context(tc.tile_pool(name="big", bufs=1))
        small = st.enter_context(tc.tile_pool(name="small", bufs=1))
        ptp = st.enter_context(tc.tile_pool(name="ptp", bufs=2 * (KT // 2)))

        # resident big tensors
        x_sb = big.tile([P, CS, NX], SDT, tag="x")         # x bf16 (partial)
        x8 = big.tile([P, CS, N], FP8, tag="x8")           # x fp8 [c, n]
        xt8 = big.tile([P, KT // 2, 2, C], FP8, tag="xt8")  # x^T fp8 pairs
        p0t = big.tile([P, CS, C], SDT, tag="p0t")         # s*Wk^T Wq (lhsT)
        p0a8 = big.tile([P, CS, C], FP8, tag="p0a8")       # A-folded fp8 P0
        w2t = big.tile([P, CS, C], SDT, tag="w2t")         # Wp@Wv (lhsT)
        qt8 = big.tile([P, CS, NQ], FP8, tag="qt8")        # QtA fp8
        cpk = const.tile([P, CPW], F32, tag="cpk")

        # ---- input DMAs: one queue (SP), priority order --------------------
        # The last quarter's stats come from x8, so its chunk goes FIRST
        # (ACT starts those reductions at ~2us) and bf16 x ships only
        # [0, NX).  cpack feeds the stats combine at ~18us.
        nc.sync.dma_start(out=x8[:, :, 3 * 1024:4 * 1024],
                          in_=x8_r[:, :, 3 * 1024:4 * 1024])
        for qd in range(NX // NCH):
            nc.sync.dma_start(out=x_sb[:, :, qd * NCH:(qd + 1) * NCH],
                              in_=x_r[:, :, qd * NCH:(qd + 1) * NCH])
        nc.sync.dma_start(out=cpk, in_=cpk_d[:, :])
        nc.sync.dma_start(out=p0t, in_=p0_r)
        # x8 and xt8 interleaved per 1024-px chunk: the S and Z streams
        # consume pixels in the same order, so each operand pair lands just
        # ahead of its first use instead of Z head-blocking on a late xt8
        for qd in range(4):
            if qd < 3:
                nc.sync.dma_start(
                    out=x8[:, :, qd * 1024:(qd + 1) * 1024],
                    in_=x8_r[:, :, qd * 1024:(qd + 1) * 1024])
            nc.sync.dma_start(
                out=xt8[:, qd * (KT // 8):(qd + 1) * (KT // 8), :, :],
                in_=xt8_d[:, qd * (KT // 8):(qd + 1) * (KT // 8), :, :])
        nc.sync.dma_start(out=w2t, in_=w2_r)

        gammaT = cpk[:, 0:CS]
        betaT = cpk[:, CS:2 * CS]
        h0T = cpk[:, 2 * CS:3 * CS]
        bp2T = cpk[:, 3 * CS:4 * CS]
        indg = cpk[:, 4 * CS:4 * CS + GPS]
        inde = cpk[0:GPS, 4 * CS + GPS:4 * CS + GPS + P]

        with ExitStack() as st1:
            ps_sm = st1.enter_context(tc.tile_pool(name="ps_sm", bufs=2,
                                                   space="PSUM"))
            ps_qt = st1.enter_context(tc.tile_pool(name="ps_qt", bufs=2,
                                                   space="PSUM"))

            nc0_t = const.tile([P, 1], F32, tag="nc0")   # exp shift (fp8 rng)
            nc.vector.memset(nc0_t, -2.5)
            ones8 = const.tile([P, 2, P], FP8, tag="ones8")  # denom lhsT
            nc.vector.memset(ones8, 1.0)
            # single ACT table load for the whole kernel: the Exp set also
            # holds Identity/Square (stats + folds); rsqrt happens on DVE
            # via Newton, so Sqrt's set is never needed.  Loading now also
            # wins the DMA-engine queue before the big input transfers.
            dume = small.tile([P, 1], F32, tag="dume")
            nc.scalar.activation(out=dume, in_=nc0_t, func=AF.Exp)

            # ---- phase 1: GroupNorm stats, pipelined with the x DMAs ------
            # Pool cannot reduce (no accum) and tensor_tensor_reduce does
            # not exist on hw.  The LAST quarter's stats come from the
            # early-DMA'd x8 chunk, entirely on ACT (Identity-accum sums +
            # Square-accum sumsq, running from ~2us while the bf16 x
            # streams).  bf16 px: DVE bn_stats on 4 windows + one double
            # window as ACT Square / DVE 4x-mode tensor_scalar sums.
            WIN_DVE = [0, 1, 2, 3]
            BF_SUM = (2048, 3072)     # bf16 dbl window 4+5
            nA = len(WIN_DVE) * NCH
            stats_all = small.tile([P, CS, len(WIN_DVE), 6], F32, tag="stats")
            sxa = small.tile([P, CS, 2, 2], F32, tag="sxa")
            scr = small.tile([P, 2, 1024], SDT, tag="scr")
            # fp8 quarter: its x8 chunk lands at ~1.6us while bf16 x is
            # still streaming, so the sum pass splits DVE (idle until the
            # first bf16 chunk) / ACT to balance the two stats queues
            for s in range(2):
                nc.vector.tensor_scalar(
                    out=scr[:, 1, :], in0=x8[:, s, 3072:4096],
                    scalar1=1.0, scalar2=0.0, op0=MUL, op1=ADD,
                    accum_out=sxa[:, s, 1, 0:1])
            for s in range(2, CS):
                nc.scalar.activation(out=scr[:, 0, :],
                                     in_=x8[:, s, 3072:4096],
                                     func=AF.Identity,
                                     accum_out=sxa[:, s, 1, 0:1])
            for s in range(CS):
                nc.scalar.activation(out=scr[:, 0, :],
                                     in_=x8[:, s, 3072:4096],
                                     func=AF.Square,
                                     accum_out=sxa[:, s, 1, 1:2])
            for wi, w0 in enumerate(WIN_DVE):
                for s in range(CS):
                    nc.vector.bn_stats(
                        out=stats_all[:, s, wi, :],
                        in_=x_sb[:, s, w0 * NCH:(w0 + 1) * NCH])
            for s in range(CS):
                nc.vector.tensor_scalar(
                    out=scr[:, 1, :], in0=x_sb[:, s, BF_SUM[0]:BF_SUM[1]],
                    scalar1=1.0, scalar2=0.0, op0=MUL, op1=ADD,
                    accum_out=sxa[:, s, 0, 0:1])
            for s in range(CS):
                nc.scalar.activation(out=scr[:, 0, :],
                                     in_=x_sb[:, s, BF_SUM[0]:BF_SUM[1]],
                                     func=AF.Square,
                                     accum_out=sxa[:, s, 0, 1:2])
            mv = small.tile([P, CS, 2], F32, tag="mv")
            for s in range(CS):
                nc.vector.bn_aggr(out=mv[:, s, :], in_=stats_all[:, s, :, :])

            # combine: the group reduction accumulates the three stats
            # sources directly in PSUM as each lands (no serial add-chain):
            # ps_g[g] = sum over sources of indg^T @ [sums | sumsqs]
            rhs8 = small.tile([P, 2 * CS], F32, tag="rhs8")
            nc.vector.tensor_scalar_mul(rhs8[:, 0:CS], mv[:, :, 0], float(nA))
            nc.vector.tensor_mul(out=rhs8[:, CS:], in0=mv[:, :, 0],
                                 in1=mv[:, :, 0])
            nc.vector.tensor_add(out=rhs8[:, CS:], in0=rhs8[:, CS:],
                                 in1=mv[:, :, 1])
            nc.vector.tensor_scalar_mul(rhs8[:, CS:], rhs8[:, CS:], float(nA))
            ps_g = ps_sm.tile([GPS, 2 * CS], F32, tag="sm", name="ps_g")
            srcs = [sxa[:, :, 1, 0], rhs8[:, 0:CS], sxa[:, :, 0, 0]]
            sqs = [sxa[:, :, 1, 1], rhs8[:, CS:], sxa[:, :, 0, 1]]
            for i in range(3):   # ordered by expected readiness
                nc.tensor.matmul(ps_g[:, 0:CS], lhsT=indg, rhs=srcs[i],
                                 start=(i == 0), stop=(i == 2),
                                 skip_group_check=True)
                nc.tensor.matmul(ps_g[:, CS:], lhsT=indg, rhs=sqs[i],
                                 start=(i == 0), stop=(i == 2),
                                 skip_group_check=True)
            gtmp = small.tile([GPS, 2 * CS], F32, tag="gtmp")
            nc.vector.tensor_scalar_mul(gtmp, ps_g, 1.0 / (cpg * N))
            # gvar = E[x^2] - mean^2 ; grstd = 1/sqrt(gvar + eps)
            gsq = small.tile([GPS, CS], F32, tag="gsq")
            nc.vector.tensor_mul(out=gsq, in0=gtmp[:, 0:CS], in1=gtmp[:, 0:CS])
            e8 = small.tile([GPS, 2 * CS], F32, tag="e8")
            wv = small.tile([GPS, CS], F32, tag="wv")
            nc.vector.scalar_tensor_tensor(   # w = (E[x^2]+eps) - mean^2
                out=wv, in0=gtmp[:, CS:], scalar=eps, in1=gsq,
                op0=ADD, op1=mybir.AluOpType.subtract)
            # rstd = rsqrt(w) by Newton on DVE (w ~ 1 for normalized input;
            # seed 1.5 - w/2 is the tangent at 1, two steps to fp32 noise)
            y_t = e8[:, 0:CS]
            nc.vector.tensor_scalar(out=y_t, in0=wv, scalar1=-0.5,
                                    scalar2=1.5, op0=MUL, op1=ADD)
            nwt = small.tile([GPS, CS], F32, tag="nwt")
            for _ in range(1):   # seed err ~4e-3 -> ~2e-5 after one step
                nc.vector.tensor_mul(out=nwt, in0=y_t, in1=y_t)
                nc.vector.tensor_mul(out=nwt, in0=nwt, in1=wv)
                nc.vector.tensor_scalar(out=nwt, in0=nwt, scalar1=-0.5,
                                        scalar2=1.5, op0=MUL, op1=ADD)
                nc.vector.tensor_mul(out=y_t, in0=y_t, in1=nwt)
            nc.vector.tensor_copy(out=e8[:, CS:], in_=gtmp[:, 0:CS])
            # expand groups -> channels
            ps_e = ps_sm.tile([P, 2 * CS], F32, tag="sm", name="ps_e")
            nc.tensor.matmul(ps_e, lhsT=inde, rhs=e8, start=True, stop=True)
            A_sb = small.tile([P, CS], F32, tag="A")     # A = gamma * rstd
            nc.vector.tensor_mul(out=A_sb, in0=ps_e[:, 0:CS], in1=gammaT)
            B32 = small.tile([P, CS], F32, tag="B32")    # B = beta - A*mean
            nc.vector.tensor_mul(out=B32, in0=ps_e[:, CS:], in1=A_sb)
            nc.vector.tensor_sub(out=B32, in0=betaT, in1=B32)
            B_sb = small.tile([P, CS], SDT, tag="B")
            nc.vector.tensor_copy(out=B_sb, in_=B32)

            # ---- phase 2: P0A fold, bias matvec, Qt production ------------
            # p0t ships pre-scaled by SC_P0 from the host, so the fold is
            # just the A row-scale; all 4 subtiles gate every Qt matmul, so
            # split it DVE/ACT (Identity is in the Exp set)
            for s in range(CS):
                if s < 2:
                    nc.vector.tensor_scalar_mul(
                        p0a8[:, s, :], p0t[:, s, :], A_sb[:, s:s + 1])
                else:
                    nc.scalar.activation(
                        out=p0a8[:, s, :], in_=p0t[:, s, :],
                        func=AF.Identity, scale=A_sb[:, s:s + 1])

            # r0 = P0^T B + h0 (Q-bias term of S, varies per key channel)
            ps_r = ps_sm.tile([P, CS], F32, tag="sm", name="ps_r")
            for cs in range(CS):
                for s in range(CS):
                    nc.tensor.matmul(
                        ps_r[:, cs:cs + 1],
                        lhsT=p0t[:, s, cs * P:(cs + 1) * P],
                        rhs=B_sb[:, s:s + 1],
                        start=(s == 0), stop=(s == CS - 1),
                        skip_group_check=True,
                    )
            # ps_r and h0T carry the host-side SC_P0 factor; the drain
            # scale/bias divide it back out
            qdr_s = small.tile([P, CS], F32, tag="qdr_s")
            nc.vector.tensor_scalar_mul(qdr_s, A_sb, SC_QT / SC_P0)
            # per-cs: the matvec is cs-major, so column 0's bias is ready
            # after 4 matmuls and the first Qt drain needn't wait for all 16
            qdr_b = small.tile([P, CS], F32, tag="qdr_b")
            for c_ in range(CS):
                nc.vector.tensor_add(out=qdr_b[:, c_:c_ + 1],
                                     in0=ps_r[:, c_:c_ + 1],
                                     in1=h0T[:, c_:c_ + 1])
                nc.vector.scalar_tensor_tensor(
                    out=qdr_b[:, c_:c_ + 1], in0=qdr_b[:, c_:c_ + 1],
                    scalar=SC_QT / SC_P0, in1=A_sb[:, c_:c_ + 1],
                    op0=MUL, op1=MUL)

            # Qt production: QtA8 = fp8(SC_QT*A_o*(psum/SC_P0 + r0_o)).
            # Only ch0 (qp0's query columns) gates the stream; it drains
            # split DVE/ACT (Identity with scale+bias APs is hw-verified).
            # ch1 is deferred into the early exp stream as PE filler.
            for cs in range(CS):
                ps_q = ps_qt.tile([P, 512], F32, tag="qt")
                for t in range(CS // 2):
                    nc.tensor.matmul(
                        ps_q,
                        lhsT=p0a8[:, 2 * t:2 * t + 2, cs * P:(cs + 1) * P],
                        rhs=x8[:, 2 * t:2 * t + 2, 0:512],
                        start=(t == 0), stop=(t == CS // 2 - 1),
                        perf_mode=mybir.MatmulPerfMode.DoubleRow,
                    )
                if cs % 2 == 0:
                    nc.vector.tensor_scalar(
                        out=qt8[:, cs, 0:512], in0=ps_q,
                        scalar1=qdr_s[:, cs:cs + 1],
                        scalar2=qdr_b[:, cs:cs + 1],
                        op0=MUL, op1=ADD,
                    )
                else:
                    nc.scalar.activation(
                        out=qt8[:, cs, 0:512], in_=ps_q, func=AF.Identity,
                        scale=qdr_s[:, cs:cs + 1],
                        bias=qdr_b[:, cs:cs + 1],
                    )

        # ---- phase 3: S -> exp -> Z0/den stream + proj + residual ---------
        with ExitStack() as st2:
            ocq = st2.enter_context(tc.tile_pool(name="ocq", bufs=2))
            outp = st2.enter_context(tc.tile_pool(name="outp", bufs=2))
            sm2 = st2.enter_context(tc.tile_pool(name="sm2", bufs=2))
            # 3 banks S stream (shared with proj psum) + 5 banks Z0/den
            ps_s = st2.enter_context(tc.tile_pool(name="ps_s", bufs=3,
                                                  space="PSUM"))
            ps_o = st2.enter_context(tc.tile_pool(name="ps_o", bufs=CS + 1,
                                                  space="PSUM"))

            bp_dev = small.tile([P, CS], F32, tag="bp")
            pt_tiles = {}

            def emit_qt_ch1(cs):
                # deferred Qt columns for qp1 (needed at g=16): fills the
                # early-stream PE gaps; drains stay off ACT's exp queue
                ps_q = ps_s.tile([P, 512], F32, tag="sbank",
                                 name=f"qt1_{cs}")
                for t in range(CS // 2):
                    nc.tensor.matmul(
                        ps_q,
                        lhsT=p0a8[:, 2 * t:2 * t + 2, cs * P:(cs + 1) * P],
                        rhs=x8[:, 2 * t:2 * t + 2, 512:1024],
                        start=(t == 0), stop=(t == CS // 2 - 1),
                        perf_mode=mybir.MatmulPerfMode.DoubleRow,
                    )
                nc.vector.tensor_scalar(
                    out=qt8[:, cs, 512:1024], in0=ps_q,
                    scalar1=qdr_s[:, cs:cs + 1],
                    scalar2=qdr_b[:, cs:cs + 1],
                    op0=MUL, op1=ADD,
                )

            def emit_s_pair(qp_, pair):
                q0_ = qp_ * QP
                pt = ptp.tile([P, 2, QP], FP8, tag="pt",
                              name=f"pt_{qp_}_{pair}")
                pt_tiles[(qp_, pair)] = pt
                for half in range(2):
                    kt = 2 * pair + half
                    s_ps = ps_s.tile([P, QP], F32, tag="sbank",
                                     name=f"s_ps_{qp_}_{kt}")
                    for t in range(CS // 2):
                        nc.tensor.matmul(
                            s_ps,
                            lhsT=x8[:, 2 * t:2 * t + 2, kt * P:(kt + 1) * P],
                            rhs=qt8[:, 2 * t:2 * t + 2, q0_:q0_ + QP],
                            start=(t == 0), stop=(t == CS // 2 - 1),
                            perf_mode=mybir.MatmulPerfMode.DoubleRow,
                        )
                    nc.scalar.activation(out=pt[:, half, :], in_=s_ps,
                                         func=AF.Exp, bias=nc0_t,
                                         scale=1.0 / SC_QT)

            def emit_z_pair(qp_, pair, o_ps, den_ps):
                pt = pt_tiles[(qp_, pair)]
                last = pair == KT // 2 - 1
                for cs in range(CS):
                    nc.tensor.matmul(
                        o_ps[cs],
                        lhsT=xt8[:, pair, :, cs * P:(cs + 1) * P],
                        rhs=pt,
                        start=(pair == 0), stop=last,
                        perf_mode=mybir.MatmulPerfMode.DoubleRow,
                    )
                nc.tensor.matmul(
                    den_ps, lhsT=ones8, rhs=pt,
                    start=(pair == 0), stop=last,
                    perf_mode=mybir.MatmulPerfMode.DoubleRow,
                )

            def emit_bp_matvec():
                # bp_dev = W2 @ B + (Wp bv + bp); w2t lands late, so this
                # slots into the qp0 stream well after the fold
                ps_z = ps_s.tile([P, CS], F32, tag="sbank", name="ps_z")
                for cs in range(CS):
                    for s in range(CS):
                        nc.tensor.matmul(
                            ps_z[:, cs:cs + 1],
                            lhsT=w2t[:, s, cs * P:(cs + 1) * P],
                            rhs=B_sb[:, s:s + 1],
                            start=(s == 0), stop=(s == CS - 1),
                            skip_group_check=True,
                        )
                nc.vector.tensor_add(out=bp_dev, in0=ps_z, in1=bp2T)

            def emit_proj_cs(qp_, cs, oc, rec_bc, ot, tt, tail=False):
                q0_ = qp_ * QP
                ps_pp = ps_s.tile([P, QP], F32, tag="sbank",
                                  name=f"pp_{qp_}_{cs}")
                for s in range(CS):
                    nc.tensor.matmul(
                        ps_pp, lhsT=w2t[:, s, cs * P:(cs + 1) * P],
                        rhs=oc[:, s, :],
                        start=(s == 0), stop=(s == CS - 1),
                    )
                # tt reads PSUM -> DVE.  Mid-stream the bias+residual adds
                # ride idle Pool (scalar_tensor_tensor is illegal there, so
                # two ops); on the latency-critical tail they spread across
                # DVE (1-op stt) and ACT(+bias)/Pool(+x).
                nc.vector.tensor_mul(out=tt[:, cs, :], in0=ps_pp, in1=rec_bc)
                if tail and cs % 2 == 1:
                    nc.vector.scalar_tensor_tensor(
                        out=ot[:, cs, :], in0=tt[:, cs, :],
                        scalar=bp_dev[:, cs:cs + 1],
                        in1=x_sb[:, cs, q0_:q0_ + QP],
                        op0=ADD, op1=ADD)
                else:
                    if tail:
                        nc.scalar.activation(out=ot[:, cs, :],
                                             in_=tt[:, cs, :],
                                             func=AF.Identity,
                                             bias=bp_dev[:, cs:cs + 1])
                    else:
                        nc.gpsimd.tensor_scalar_add(ot[:, cs, :],
                                                    tt[:, cs, :],
                                                    bp_dev[:, cs:cs + 1])
                    nc.gpsimd.tensor_add(out=ot[:, cs, :], in0=ot[:, cs, :],
                                         in1=x_sb[:, cs, q0_:q0_ + QP])
                # out rides SP while streaming (a waiting dma_start holds its
                # engine's SEQ); on the tail ACT is free and shares the load
                eng = nc.scalar if (tail and cs % 2 == 1) else nc.sync
                eng.dma_start(
                    out=out_r[:, cs, q0_:q0_ + QP], in_=ot[:, cs, :])

            def finish_qpass(qp, o_ps, den_ps):
                # Z0 drains first (they gate the proj matmuls); the den
                # reciprocal is only needed ~3us later by tt
                oc = ocq.tile([P, CS, QP], SDT, tag="ocq")
                for cs in range(CS):
                    # mid-stream qpasses keep ACT free for exp; the final
                    # qpass's drains split DVE/ACT (exp stream is over)
                    if qp < QPASSES - 1 or cs % 2 == 0:
                        nc.vector.tensor_scalar_mul(oc[:, cs, :], o_ps[cs],
                                                    A_sb[:, cs:cs + 1])
                    else:
                        nc.scalar.activation(out=oc[:, cs, :], in_=o_ps[cs],
                                             func=AF.Identity,
                                             scale=A_sb[:, cs:cs + 1])
                rec_bc = sm2.tile([P, QP], F32, tag="recbc", name=f"rb_{qp}")
                nc.vector.reciprocal(out=rec_bc, in_=den_ps)
                ot = outp.tile([P, CS, QP], SDT, tag="ot")
                tt = outp.tile([P, CS, QP], F32, tag="tt")
                return (oc, rec_bc, ot, tt)

            # One global Z stream trailing the S stream by ZLAG pairs: it
            # rides out the late xt8 DMA arrival, keeps PE fed while ACT
            # exps, and crosses qpass boundaries without stalling (qp0's Z
            # tail drains inside qp1's S stream).  The previous qpass's
            # proj chains then slot into PE gaps ~4 pairs later, once its
            # Z0 drains have cleared DVE.
            NP2 = KT // 2
            all_pairs = [(qp, pair) for qp in range(QPASSES)
                         for pair in range(NP2)]
            state = {}
            z_idx = 0
            prev = None
            prev_qp = -1
            prev_age = 0
            projs_done = CS
            for g, (qp, pair) in enumerate(all_pairs):
                if pair == 0:
                    o_ps = [ps_o.tile([P, QP], F32, tag="o",
                                      name=f"o_{qp}_{c}") for c in range(CS)]
                    den_ps = ps_o.tile([P, QP], F32, tag="o",
                                       name=f"den_{qp}")
                    state[qp] = (o_ps, den_ps)
                emit_s_pair(qp, pair)
                zlag = 1
                budget = 2
                while z_idx <= g - zlag and budget > 0:
                    zq, zp = all_pairs[z_idx]
                    emit_z_pair(zq, zp, *state[zq])
                    z_idx += 1
                    budget -= 1
                    if zp == NP2 - 1:
                        prev = finish_qpass(zq, *state[zq])
                        prev_qp = zq
                        prev_age = 0
                        projs_done = 0
                if qp == 0 and pair in (1, 2, 3, 4):
                    emit_qt_ch1(pair - 1)
                if qp == 0 and pair == 7:
                    emit_bp_matvec()
                prev_age += 1
                if (prev is not None and projs_done < CS and prev_age >= 4
                        and prev_age % 2 == 0):
                    emit_proj_cs(prev_qp, projs_done, *prev)
                    projs_done += 1
            while z_idx < len(all_pairs):
                zq, zp = all_pairs[z_idx]
                emit_z_pair(zq, zp, *state[zq])
                z_idx += 1
                if zp == NP2 - 1:
                    prev = finish_qpass(zq, *state[zq])
                    prev_qp = zq
                    projs_done = 0
            for cs in range(projs_done, CS):
                emit_proj_cs(prev_qp, cs, *prev, tail=True)

    nc.finalize()
    return nc


def make_consts(P=128, cpg=16):
    GPS = P // cpg
    indg = np.zeros((P, GPS), np.float32)
    for p in range(P):
        indg[p, p // cpg] = 1.0
    inde = indg.T.copy()
    return indg, inde


_PROGRAM_CACHE = {}


def _get_program(C, G, N, NQ, precision="tf32"):
    key = (C, G, N, NQ, precision)
    if key not in _PROGRAM_CACHE:
        _PROGRAM_CACHE[key] = build_program(C=C, G=G, N=N, NQ=NQ,
                                            precision=precision)
    return _PROGRAM_CACHE[key]


def make_in_maps(x, gn_w, gn_b, q_w, q_b, k_w, k_b, v_w, v_b, proj_w, proj_b,
                 n_cores=8, G=32):
    """Shard full inputs into per-core input maps (weight products folded on
    host).  Per-core x is pixel-permuted so the core's query block is first;
    attention is permutation-invariant over keys so S/Z stay consistent."""
    import ml_dtypes
    bf = ml_dtypes.bfloat16
    f8 = ml_dtypes.float8_e4m3
    f = lambda a: np.ascontiguousarray(np.asarray(a, dtype=np.float32))
    x = f(x)
    b, c, h, w = x.shape
    n = h * w
    qblocks = n_cores // b
    nq = n // qblocks
    cs = c // 128
    kt = n // 128
    gps = 128 // (c // G)
    scale = np.float32(c ** -0.5)
    xf = x.reshape(b, c, n)

    def to_pcs(v):                       # [C] -> [128, CS] (c = 128*s + p)
        return np.asarray(v, np.float32).reshape(cs, 128).T

    qw, kw, vw, pw = f(q_w), f(k_w), f(v_w), f(proj_w)
    indg, inde = make_consts(cpg=c // G)
    cpk = np.zeros((128, 4 * cs + gps + 128), np.float32)
    cpk[:, 0:cs] = to_pcs(f(gn_w))
    cpk[:, cs:2 * cs] = to_pcs(f(gn_b))
    cpk[:, 2 * cs:3 * cs] = to_pcs(64.0 * scale * (kw.T @ f(q_b)))
    cpk[:, 3 * cs:4 * cs] = to_pcs(pw @ f(v_b) + f(proj_b))
    cpk[:, 4 * cs:4 * cs + gps] = indg
    cpk[0:gps, 4 * cs + gps:] = inde
    common = {
        # pre-scaled by SC_P0=64: the device fold is then just the A
        # row-scale, and drains divide the 64 back out
        "p0t": np.ascontiguousarray((64.0 * scale * (qw.T @ kw)).astype(bf)),
        "w2t": np.ascontiguousarray((pw @ vw).T.astype(bf)),
        "cpk": cpk,
    }
    in_maps = []
    for i in range(n_cores):
        bi, qi = divmod(i, qblocks)
        xb = xf[bi]
        qs, qe = qi * nq, (qi + 1) * nq
        xperm = np.concatenate([xb[:, qs:qe], xb[:, :qs], xb[:, qe:]], axis=1)
        x8 = xperm.astype(f8)
        xt8 = np.ascontiguousarray(
            x8.T.reshape(kt // 2, 2, 128, c).transpose(2, 0, 1, 3))
        in_maps.append({
            **common,
            # bf16 x ships only where bf16 stats windows + residual read it;
            # the last quarter's stats come from x8 on device
            "x": np.ascontiguousarray(xperm[:, :n - 1024].astype(bf)),
            "x8": np.ascontiguousarray(x8),
            "xt8": xt8,
        })
    return in_maps, (b, c, h, w, n, nq, qblocks)


def kernel(x, gn_w, gn_b, q_w, q_b, k_w, k_b, v_w, v_b, proj_w, proj_b):
    from concourse.bass_utils import run_bass_kernel_spmd

    in_maps, (b, c, h, w, n, nq, qblocks) = make_in_maps(
        x, gn_w, gn_b, q_w, q_b, k_w, k_b, v_w, v_b, proj_w, proj_b
    )
    n_cores = 8
    nc = _get_program(C=c, G=32, N=n, NQ=nq)
    res = run_bass_kernel_spmd(nc, in_maps, list(range(n_cores))).results
    out = np.empty((b, c, n), np.float32)
    for i in range(n_cores):
        bi, qi = divmod(i, qblocks)
        out[bi, :, qi * nq:(qi + 1) * nq] = res[i]["out"]
    return out.reshape(b, c, h, w)


# revision 75
# speedup vs baseline: 1.7468x; 1.7468x over previous
"""AttnBlock (GroupNorm -> QKV 1x1 -> full NxN attention -> proj -> residual)
for Trainium2, SPMD over 8 NeuronCores.

Sharding: data-parallel over batch (2) x query-pixel blocks (4 of 1024 px).
Each core receives its batch image x [C, N] PERMUTED so that its own query
block occupies pixels [0, NQ); attention is permutation-invariant over keys.
No collectives.

v3 structure — K and V are never materialized.  Weight products fold on the
host; GroupNorm folds into tiny per-channel vectors on device:

  hn = A*x + B (per-channel).  With P0 = s*Wk^T@Wq and W2 = Wp@Wv (host):
    S[k,q]  = x[:,k]^T @ QtA[:,q]   (+ per-q consts that cancel in softmax)
    QtA     = diag(A) (P0^T diag(A) x_q + P0^T B + s Wk^T bq)
    out     = W2 (A . Z0) / den + (W2 B + Wp bv + bp) + x,   Z0 = x @ P^T

  Device tensors: x bf16 (stats + residual), x8 fp8 [c,n] (S lhsT + query
  rhs), xt8 fp8 [k,c] pair-interleaved (Z lhsT; host-transposed), p0t bf16,
  w2t bf16, one packed const vector.  GroupNorm A enters via a [C,C] lhsT
  row-scale (P0A8 fold), a drain scale on Qt, and a drain scale on Z0; all
  B / bias terms ride along as drain biases or the proj bias.

  Heavy matmuls (Qt production, S, Z0, softmax denominators) run fp8 e4m3
  DoubleRow (0.5 cyc/row); proj runs bf16.

  Scheduling notes (cost-model driven):
  - All input DMAs ride the SP queue in priority order (p0t, x, x8, xt8,
    w2t); the DMA engines are a single ~25us serial resource, so order is
    everything.  Stats windows pipeline with the x chunks as they land.
  - Per 512-px window: DVE does the sums (tensor_scalar+accum runs 4x on
    bf16) + one sumsq; ACT two sumsq (Square+accum); Pool one sumsq.
  - ACT loads the Sqrt table set first (it also holds Identity/Square for
    the stats window), switches to the Exp set once, then owns the exp
    stream; every other drain lives on DVE/Pool.
  - S->exp->Z0 pipeline: S pairs on PE feed ACT exp; Z0/den DoubleRow
    matmuls trail one pair behind; the previous qpass's proj (and the
    bp_dev matvec) slot into PE gaps of the exp-bound stream.
"""

from contextlib import ExitStack

import numpy as np

import concourse.bacc as bacc
import concourse.bass as bass
import concourse.mybir as mybir
import concourse.tile as tile

F32 = mybir.dt.float32
F32R = mybir.dt.float32r
BF16 = mybir.dt.bfloat16
FP8 = mybir.dt.float8e4
AF = mybir.ActivationFunctionType
MUL = mybir.AluOpType.mult
ADD = mybir.AluOpType.add

SC_P0 = 64.0      # fp8 P0A lhsT pre-scale (dodges e4m3 subnormals)
SC_QT = 16.0      # fp8 Qt storage scale


def build_program(C=512, G=32, N=4096, NQ=1024, eps=1e-5, precision="tf32"):
    """Emit the per-core Bass program (SPMD; per-core data differs only)."""
    P = 128
    CS = C // P                  # channel subtiles
    KT = N // P                  # key/pixel tiles
    NCH = 512                    # x DMA chunk / stats window (px)
    NCHUNKS = N // NCH
    QP = min(512, NQ)            # query-pass width
    QPASSES = NQ // QP
    cpg = C // G                 # channels per group
    GPS = P // cpg               # groups per channel-subtile
    assert C % P == 0 and N % P == 0 and NQ % QP == 0 and P % cpg == 0
    SDT = BF16

    nc = bacc.Bacc(None, target_bir_lowering=False)

    NX = N - 1024        # bf16 x ships only for stats windows + residual;
    x_d = nc.dram_tensor("x", [C, NX], SDT, kind="ExternalInput")
    x8_d = nc.dram_tensor("x8", [C, N], FP8, kind="ExternalInput")
    xt8_d = nc.dram_tensor("xt8", [P, KT // 2, 2, C], FP8, kind="ExternalInput")
    p0t_d = nc.dram_tensor("p0t", [C, C], SDT, kind="ExternalInput")
    w2t_d = nc.dram_tensor("w2t", [C, C], SDT, kind="ExternalInput")
    CPW = 4 * CS + GPS + P
    cpk_d = nc.dram_tensor("cpk", [P, CPW], F32, kind="ExternalInput")
    out_d = nc.dram_tensor("out", [C, NQ], SDT, kind="ExternalOutput")

    x_r = x_d[:, :].rearrange("(s p) n -> p s n", p=P)
    x8_r = x8_d[:, :].rearrange("(s p) n -> p s n", p=P)
    p0_r = p0t_d[:, :].rearrange("(s p) o -> p s o", p=P)
    w2_r = w2t_d[:, :].rearrange("(s p) o -> p s o", p=P)
    out_r = out_d[:, :].rearrange("(s p) n -> p s n", p=P)

    with tile.TileContext(nc) as tc, ExitStack() as st:
        const = st.enter_context(tc.tile_pool(name="const", bufs=1))
        big = st.enter_context(tc.tile_pool(name="big", bufs=1))
        small = st.enter_context(tc.tile_pool(name="small", bufs=1))
        ptp = st.enter_context(tc.tile_pool(name="ptp", bufs=2 * (KT // 2)))

        # resident big tensors
        x_sb = big.tile([P, CS, NX], SDT, tag="x")         # x bf16 (partial)
        x8 = big.tile([P, CS, N], FP8, tag="x8")           # x fp8 [c, n]
        xt8 = big.tile([P, KT // 2, 2, C], FP8, tag="xt8")  # x^T fp8 pairs
        p0t = big.tile([P, CS, C], SDT, tag="p0t")         # s*Wk^T Wq (lhsT)
        p0a8 = big.tile([P, CS, C], FP8, tag="p0a8")       # A-folded fp8 P0
        w2t = big.tile([P, CS, C], SDT, tag="w2t")         # Wp@Wv (lhsT)
        qt8 = big.tile([P, CS, NQ], FP8, tag="qt8")        # QtA fp8
        cpk = const.tile([P, CPW], F32, tag="cpk")

        # ---- input DMAs: one queue (SP), priority order --------------------
        # The last quarter's stats come from x8, so its chunk goes FIRST
        # (ACT starts those reductions at ~2us) and bf16 x ships only
        # [0, NX).  cpack feeds the stats combine at ~18us.
        nc.sync.dma_start(out=x8[:, :, 3 * 1024:4 * 1024],
                          in_=x8_r[:, :, 3 * 1024:4 * 1024])
        for qd in range(NX // NCH):
            nc.sync.dma_start(out=x_sb[:, :, qd * NCH:(qd + 1) * NCH],
                              in_=x_r[:, :, qd * NCH:(qd + 1) * NCH])
        nc.sync.dma_start(out=cpk, in_=cpk_d[:, :])
        nc.sync.dma_start(out=p0t, in_=p0_r)
        # x8 and xt8 interleaved per 1024-px chunk: the S and Z streams
        # consume pixels in the same order, so each operand pair lands just
        # ahead of its first use instead of Z head-blocking on a late xt8
        for qd in range(4):
            if qd < 3:
                nc.sync.dma_start(
                    out=x8[:, :, qd * 1024:(qd + 1) * 1024],
                    in_=x8_r[:, :, qd * 1024:(qd + 1) * 1024])
            nc.sync.dma_start(
                out=xt8[:, qd * (KT // 8):(qd + 1) * (KT // 8), :, :],
                in_=xt8_d[:, qd * (KT // 8):(qd + 1) * (KT // 8), :, :])
        nc.sync.dma_start(out=w2t, in_=w2_r)

        gammaT = cpk[:, 0:CS]
        betaT = cpk[:, CS:2 * CS]
        h0T = cpk[:, 2 * CS:3 * CS]
        bp2T = cpk[:, 3 * CS:4 * CS]
        indg = cpk[:, 4 * CS:4 * CS + GPS]
        inde = cpk[0:GPS, 4 * CS + GPS:4 * CS + GPS + P]

        with ExitStack() as st1:
            ps_sm = st1.enter_context(tc.tile_pool(name="ps_sm", bufs=2,
                                                   space="PSUM"))
            ps_qt = st1.enter_context(tc.tile_pool(name="ps_qt", bufs=2,
                                                   space="PSUM"))

            nc0_t = const.tile([P, 1], F32, tag="nc0")   # exp shift (fp8 rng)
            nc.vector.memset(nc0_t, -2.5)
            ones8 = const.tile([P, 2, P], FP8, tag="ones8")  # denom lhsT
            nc.vector.memset(ones8, 1.0)
            # single ACT table load for the whole kernel: the Exp set also
            # holds Identity/Square (stats + folds); rsqrt happens on DVE
            # via Newton, so Sqrt's set is never needed.  Loading now also
            # wins the DMA-engine queue before the big input transfers.
            dume = small.tile([P, 1], F32, tag="dume")
            nc.scalar.activation(out=dume, in_=nc0_t, func=AF.Exp)

            # ---- phase 1: GroupNorm stats, pipelined with the x DMAs ------
            # Pool cannot reduce (no accum) and tensor_tensor_reduce does
            # not exist on hw.  The LAST quarter's stats come from the
            # early-DMA'd x8 chunk, entirely on ACT (Identity-accum sums +
            # Square-accum sumsq, running from ~2us while the bf16 x
            # streams).  bf16 px: DVE bn_stats on 4 windows + one double
            # window as ACT Square / DVE 4x-mode tensor_scalar sums.
            WIN_DVE = [0, 1, 2, 3]
            BF_SUM = (2048, 3072)     # bf16 dbl window 4+5
            nA = len(WIN_DVE) * NCH
            stats_all = small.tile([P, CS, len(WIN_DVE), 6], F32, tag="stats")
            sxa = small.tile([P, CS, 2, 2], F32, tag="sxa")
            scr = small.tile([P, 2, 1024], SDT, tag="scr")
            # fp8 quarter: its x8 chunk lands at ~1.6us while bf16 x is
            # still streaming, so the sum pass splits DVE (idle until the
            # first bf16 chunk) / ACT to balance the two stats queues
            for s in range(2):
                nc.vector.tensor_scalar(
                    out=scr[:, 1, :], in0=x8[:, s, 3072:4096],
                    scalar1=1.0, scalar2=0.0, op0=MUL, op1=ADD,
                    accum_out=sxa[:, s, 1, 0:1])
            for s in range(2, CS):
                nc.scalar.activation(out=scr[:, 0, :],
                                     in_=x8[:, s, 3072:4096],
                                     func=AF.Identity,
                                     accum_out=sxa[:, s, 1, 0:1])
            for s in range(CS):
                nc.scalar.activation(out=scr[:, 0, :],
                                     in_=x8[:, s, 3072:4096],
                                     func=AF.Square,
                                     accum_out=sxa[:, s, 1, 1:2])
            for wi, w0 in enumerate(WIN_DVE):
                for s in range(CS):
                    nc.vector.bn_stats(
                        out=stats_all[:, s, wi, :],
                        in_=x_sb[:, s, w0 * NCH:(w0 + 1) * NCH])
            for s in range(CS):
                nc.vector.tensor_scalar(
                    out=scr[:, 1, :], in0=x_sb[:, s, BF_SUM[0]:BF_SUM[1]],
                    scalar1=1.0, scalar2=0.0, op0=MUL, op1=ADD,
                    accum_out=sxa[:, s, 0, 0:1])
            for s in range(CS):
                nc.scalar.activation(out=scr[:, 0, :],
                                     in_=x_sb[:, s, BF_SUM[0]:BF_SUM[1]],
                                     func=AF.Square,
                                     accum_out=sxa[:, s, 0, 1:2])
            mv = small.tile([P, CS, 2], F32, tag="mv")
            for s in range(CS):
                nc.vector.bn_aggr(out=mv[:, s, :], in_=stats_all[:, s, :, :])

            # combine: the group reduction accumulates the three stats
            # sources directly in PSUM as each lands (no serial add-chain):
            # ps_g[g] = sum over sources of indg^T @ [sums | sumsqs]
            rhs8 = small.tile([P, 2 * CS], F32, tag="rhs8")
            nc.vector.tensor_scalar_mul(rhs8[:, 0:CS], mv[:, :, 0], float(nA))
            nc.vector.tensor_mul(out=rhs8[:, CS:], in0=mv[:, :, 0],
                                 in1=mv[:, :, 0])
            nc.vector.tensor_add(out=rhs8[:, CS:], in0=rhs8[:, CS:],
                                 in1=mv[:, :, 1])
            nc.vector.tensor_scalar_mul(rhs8[:, CS:], rhs8[:, CS:], float(nA))
            ps_g = ps_sm.tile([GPS, 2 * CS], F32, tag="sm", name="ps_g")
            srcs = [sxa[:, :, 1, 0], rhs8[:, 0:CS], sxa[:, :, 0, 0]]
            sqs = [sxa[:, :, 1, 1], rhs8[:, CS:], sxa[:, :, 0, 1]]
            for i in range(3):   # ordered by expected readiness
                nc.tensor.matmul(ps_g[:, 0:CS], lhsT=indg, rhs=srcs[i],
                                 start=(i == 0), stop=(i == 2),
                                 skip_group_check=True)
                nc.tensor.matmul(ps_g[:, CS:], lhsT=indg, rhs=sqs[i],
                                 start=(i == 0), stop=(i == 2),
                                 skip_group_check=True)
            gtmp = small.tile([GPS, 2 * CS], F32, tag="gtmp")
            nc.vector.tensor_scalar_mul(gtmp, ps_g, 1.0 / (cpg * N))
            # gvar = E[x^2] - mean^2 ; grstd = 1/sqrt(gvar + eps)
            gsq = small.tile([GPS, CS], F32, tag="gsq")
            nc.vector.tensor_mul(out=gsq, in0=gtmp[:, 0:CS], in1=gtmp[:, 0:CS])
            e8 = small.tile([GPS, 2 * CS], F32, tag="e8")
            wv = small.tile([GPS, CS], F32, tag="wv")
            nc.vector.scalar_tensor_tensor(   # w = (E[x^2]+eps) - mean^2
                out=wv, in0=gtmp[:, CS:], scalar=eps, in1=gsq,
                op0=ADD, op1=mybir.AluOpType.subtract)
            # rstd = rsqrt(w) by Newton on DVE (w ~ 1 for normalized input;
            # seed 1.5 - w/2 is the tangent at 1, two steps to fp32 noise)
            y_t = e8[:, 0:CS]
            nc.vector.tensor_scalar(out=y_t, in0=wv, scalar1=-0.5,
                                    scalar2=1.5, op0=MUL, op1=ADD)
            nwt = small.tile([GPS, CS], F32, tag="nwt")
            for _ in range(1):   # seed err ~4e-3 -> ~2e-5 after one step
                nc.vector.tensor_mul(out=nwt, in0=y_t, in1=y_t)
                nc.vector.tensor_mul(out=nwt, in0=nwt, in1=wv)
                nc.vector.tensor_scalar(out=nwt, in0=nwt, scalar1=-0.5,
                                        scalar2=1.5, op0=MUL, op1=ADD)
                nc.vector.tensor_mul(out=y_t, in0=y_t, in1=nwt)
            nc.vector.tensor_copy(out=e8[:, CS:], in_=gtmp[:, 0:CS])
            # expand groups -> channels
            ps_e = ps_sm.tile([P, 2 * CS], F32, tag="sm", name="ps_e")
            nc.tensor.matmul(ps_e, lhsT=inde, rhs=e8, start=True, stop=True)
            A_sb = small.tile([P, CS], F32, tag="A")     # A = gamma * rstd
            nc.vector.tensor_mul(out=A_sb, in0=ps_e[:, 0:CS], in1=gammaT)
            B32 = small.tile([P, CS], F32, tag="B32")    # B = beta - A*mean
            nc.vector.tensor_mul(out=B32, in0=ps_e[:, CS:], in1=A_sb)
            nc.vector.tensor_sub(out=B32, in0=betaT, in1=B32)
            B_sb = small.tile([P, CS], SDT, tag="B")
            nc.vector.tensor_copy(out=B_sb, in_=B32)

            # ---- phase 2: P0A fold, bias matvec, Qt production ------------
            # p0t ships pre-scaled by SC_P0 from the host, so the fold is
            # just the A row-scale; all 4 subtiles gate every Qt matmul, so
            # split it DVE/ACT (Identity is in the Exp set)
            # three-engine fold (Pool's AP tensor_scalar_mul is the proven
            # baseline wv8-fold form): wall ~0.9us instead of two rounds
            FOLD_ENG = {0: nc.vector, 3: nc.vector, 1: nc.gpsimd}
            for s in range(CS):
                if s == 2:
                    nc.scalar.activation(
                        out=p0a8[:, s, :], in_=p0t[:, s, :],
                        func=AF.Identity, scale=A_sb[:, s:s + 1])
                else:
                    FOLD_ENG[s].tensor_scalar_mul(
                        p0a8[:, s, :], p0t[:, s, :], A_sb[:, s:s + 1])

            # r0 = P0^T B + h0 (Q-bias term of S, varies per key channel)
            ps_r = ps_sm.tile([P, CS], F32, tag="sm", name="ps_r")
            for cs in range(CS):
                for s in range(CS):
                    nc.tensor.matmul(
                        ps_r[:, cs:cs + 1],
                        lhsT=p0t[:, s, cs * P:(cs + 1) * P],
                        rhs=B_sb[:, s:s + 1],
                        start=(s == 0), stop=(s == CS - 1),
                        skip_group_check=True,
                    )
            # ps_r and h0T carry the host-side SC_P0 factor; the drain
            # scale/bias divide it back out
            qdr_s = small.tile([P, CS], F32, tag="qdr_s")
            nc.vector.tensor_scalar_mul(qdr_s, A_sb, SC_QT / SC_P0)
            # per-cs: the matvec is cs-major, so column 0's bias is ready
            # after 4 matmuls and the first Qt drain needn't wait for all 16
            qdr_b = small.tile([P, CS], F32, tag="qdr_b")
            for c_ in range(CS):
                nc.vector.tensor_add(out=qdr_b[:, c_:c_ + 1],
                                     in0=ps_r[:, c_:c_ + 1],
                                     in1=h0T[:, c_:c_ + 1])
                nc.vector.scalar_tensor_tensor(
                    out=qdr_b[:, c_:c_ + 1], in0=qdr_b[:, c_:c_ + 1],
                    scalar=SC_QT / SC_P0, in1=A_sb[:, c_:c_ + 1],
                    op0=MUL, op1=MUL)

            # Qt production: QtA8 = fp8(SC_QT*A_o*(psum/SC_P0 + r0_o)).
            # Only ch0 (qp0's query columns) gates the stream; it drains
            # split DVE/ACT (Identity with scale+bias APs is hw-verified).
            # ch1 is deferred into the early exp stream as PE filler.
            for cs in range(CS):
                ps_q = ps_qt.tile([P, 512], F32, tag="qt")
                for t in range(CS // 2):
                    nc.tensor.matmul(
                        ps_q,
                        lhsT=p0a8[:, 2 * t:2 * t + 2, cs * P:(cs + 1) * P],
                        rhs=x8[:, 2 * t:2 * t + 2, 0:512],
                        start=(t == 0), stop=(t == CS // 2 - 1),
                        perf_mode=mybir.MatmulPerfMode.DoubleRow,
                    )
                if cs % 2 == 0:
                    nc.vector.tensor_scalar(
                        out=qt8[:, cs, 0:512], in0=ps_q,
                        scalar1=qdr_s[:, cs:cs + 1],
                        scalar2=qdr_b[:, cs:cs + 1],
                        op0=MUL, op1=ADD,
                    )
                else:
                    nc.scalar.activation(
                        out=qt8[:, cs, 0:512], in_=ps_q, func=AF.Identity,
                        scale=qdr_s[:, cs:cs + 1],
                        bias=qdr_b[:, cs:cs + 1],
                    )

        # ---- phase 3: S -> exp -> Z0/den stream + proj + residual ---------
        with ExitStack() as st2:
            ocq = st2.enter_context(tc.tile_pool(name="ocq", bufs=2))
            outp = st2.enter_context(tc.tile_pool(name="outp", bufs=2))
            sm2 = st2.enter_context(tc.tile_pool(name="sm2", bufs=2))
            # 3 banks S stream (shared with proj psum) + 5 banks Z0/den
            ps_s = st2.enter_context(tc.tile_pool(name="ps_s", bufs=3,
                                                  space="PSUM"))
            ps_o = st2.enter_context(tc.tile_pool(name="ps_o", bufs=CS + 1,
                                                  space="PSUM"))

            bp_dev = small.tile([P, CS], F32, tag="bp")
            pt_tiles = {}

            def emit_qt_ch1(cs):
                # deferred Qt columns for qp1 (needed at g=16): fills the
                # early-stream PE gaps; drains stay off ACT's exp queue
                ps_q = ps_s.tile([P, 512], F32, tag="sbank",
                                 name=f"qt1_{cs}")
                for t in range(CS // 2):
                    nc.tensor.matmul(
                        ps_q,
                        lhsT=p0a8[:, 2 * t:2 * t + 2, cs * P:(cs + 1) * P],
                        rhs=x8[:, 2 * t:2 * t + 2, 512:1024],
                        start=(t == 0), stop=(t == CS // 2 - 1),
                        perf_mode=mybir.MatmulPerfMode.DoubleRow,
                    )
                nc.vector.tensor_scalar(
                    out=qt8[:, cs, 512:1024], in0=ps_q,
                    scalar1=qdr_s[:, cs:cs + 1],
                    scalar2=qdr_b[:, cs:cs + 1],
                    op0=MUL, op1=ADD,
                )

            def emit_s_pair(qp_, pair):
                q0_ = qp_ * QP
                pt = ptp.tile([P, 2, QP], FP8, tag="pt",
                              name=f"pt_{qp_}_{pair}")
                pt_tiles[(qp_, pair)] = pt
                for half in range(2):
                    kt = 2 * pair + half
                    s_ps = ps_s.tile([P, QP], F32, tag="sbank",
                                     name=f"s_ps_{qp_}_{kt}")
                    for t in range(CS // 2):
                        nc.tensor.matmul(
                            s_ps,
                            lhsT=x8[:, 2 * t:2 * t + 2, kt * P:(kt + 1) * P],
                            rhs=qt8[:, 2 * t:2 * t + 2, q0_:q0_ + QP],
                            start=(t == 0), stop=(t == CS // 2 - 1),
                            perf_mode=mybir.MatmulPerfMode.DoubleRow,
                        )
                    nc.scalar.activation(out=pt[:, half, :], in_=s_ps,
                                         func=AF.Exp, bias=nc0_t,
                                         scale=1.0 / SC_QT)

            def emit_z_pair(qp_, pair, o_ps, den_ps):
                pt = pt_tiles[(qp_, pair)]
                last = pair == KT // 2 - 1
                for cs in range(CS):
                    nc.tensor.matmul(
                        o_ps[cs],
                        lhsT=xt8[:, pair, :, cs * P:(cs + 1) * P],
                        rhs=pt,
                        start=(pair == 0), stop=last,
                        perf_mode=mybir.MatmulPerfMode.DoubleRow,
                    )
                nc.tensor.matmul(
                    den_ps, lhsT=ones8, rhs=pt,
                    start=(pair == 0), stop=last,
                    perf_mode=mybir.MatmulPerfMode.DoubleRow,
                )

            def emit_bp_matvec():
                # bp_dev = W2 @ B + (Wp bv + bp); w2t lands late, so this
                # slots into the qp0 stream well after the fold
                ps_z = ps_s.tile([P, CS], F32, tag="sbank", name="ps_z")
                for cs in range(CS):
                    for s in range(CS):
                        nc.tensor.matmul(
                            ps_z[:, cs:cs + 1],
                            lhsT=w2t[:, s, cs * P:(cs + 1) * P],
                            rhs=B_sb[:, s:s + 1],
                            start=(s == 0), stop=(s == CS - 1),
                            skip_group_check=True,
                        )
                nc.vector.tensor_add(out=bp_dev, in0=ps_z, in1=bp2T)

            def emit_proj_cs(qp_, cs, oc, rec_bc, ot, tt, tail=False):
                q0_ = qp_ * QP
                ps_pp = ps_s.tile([P, QP], F32, tag="sbank",
                                  name=f"pp_{qp_}_{cs}")
                for s in range(CS):
                    nc.tensor.matmul(
                        ps_pp, lhsT=w2t[:, s, cs * P:(cs + 1) * P],
                        rhs=oc[:, s, :],
                        start=(s == 0), stop=(s == CS - 1),
                    )
                # tt reads PSUM -> DVE.  Mid-stream the bias+residual adds
                # ride idle Pool (scalar_tensor_tensor is illegal there, so
                # two ops); on the latency-critical tail they spread across
                # DVE (1-op stt) and ACT(+bias)/Pool(+x).
                nc.vector.tensor_mul(out=tt[:, cs, :], in0=ps_pp, in1=rec_bc)
                if tail and cs % 2 == 1:
                    nc.vector.scalar_tensor_tensor(
                        out=ot[:, cs, :], in0=tt[:, cs, :],
                        scalar=bp_dev[:, cs:cs + 1],
                        in1=x_sb[:, cs, q0_:q0_ + QP],
                        op0=ADD, op1=ADD)
                else:
                    if tail:
                        nc.scalar.activation(out=ot[:, cs, :],
                                             in_=tt[:, cs, :],
                                             func=AF.Identity,
                                             bias=bp_dev[:, cs:cs + 1])
                    else:
                        nc.gpsimd.tensor_scalar_add(ot[:, cs, :],
                                                    tt[:, cs, :],
                                                    bp_dev[:, cs:cs + 1])
                    nc.gpsimd.tensor_add(out=ot[:, cs, :], in0=ot[:, cs, :],
                                         in1=x_sb[:, cs, q0_:q0_ + QP])
                # out rides SP while streaming (a waiting dma_start holds its
                # engine's SEQ); on the tail ACT is free and shares the load
                eng = nc.scalar if (tail and cs % 2 == 1) else nc.sync
                eng.dma_start(
                    out=out_r[:, cs, q0_:q0_ + QP], in_=ot[:, cs, :])

            def finish_qpass(qp, o_ps, den_ps):
                # Z0 drains first (they gate the proj matmuls); the den
                # reciprocal is only needed ~3us later by tt
                oc = ocq.tile([P, CS, QP], SDT, tag="ocq")
                for cs in range(CS):
                    # mid-stream qpasses keep ACT free for exp; the final
                    # qpass's drains split DVE/ACT (exp stream is over)
                    if qp < QPASSES - 1 or cs % 2 == 0:
                        nc.vector.tensor_scalar_mul(oc[:, cs, :], o_ps[cs],
                                                    A_sb[:, cs:cs + 1])
                    else:
                        nc.scalar.activation(out=oc[:, cs, :], in_=o_ps[cs],
                                             func=AF.Identity,
                                             scale=A_sb[:, cs:cs + 1])
                rec_bc = sm2.tile([P, QP], F32, tag="recbc", name=f"rb_{qp}")
                nc.vector.reciprocal(out=rec_bc, in_=den_ps)
                ot = outp.tile([P, CS, QP], SDT, tag="ot")
                tt = outp.tile([P, CS, QP], F32, tag="tt")
                return (oc, rec_bc, ot, tt)

            # One global Z stream trailing the S stream by ZLAG pairs: it
            # rides out the late xt8 DMA arrival, keeps PE fed while ACT
            # exps, and crosses qpass boundaries without stalling (qp0's Z
            # tail drains inside qp1's S stream).  The previous qpass's
            # proj chains then slot into PE gaps ~4 pairs later, once its
            # Z0 drains have cleared DVE.
            NP2 = KT // 2
            all_pairs = [(qp, pair) for qp in range(QPASSES)
                         for pair in range(NP2)]
            state = {}
            z_idx = 0
            prev = None
            prev_qp = -1
            prev_age = 0
            projs_done = CS
            for g, (qp, pair) in enumerate(all_pairs):
                if pair == 0:
                    o_ps = [ps_o.tile([P, QP], F32, tag="o",
                                      name=f"o_{qp}_{c}") for c in range(CS)]
                    den_ps = ps_o.tile([P, QP], F32, tag="o",
                                       name=f"den_{qp}")
                    state[qp] = (o_ps, den_ps)
                emit_s_pair(qp, pair)
                zlag = 1
                budget = 2
                while z_idx <= g - zlag and budget > 0:
                    zq, zp = all_pairs[z_idx]
                    emit_z_pair(zq, zp, *state[zq])
                    z_idx += 1
                    budget -= 1
                    if zp == NP2 - 1:
                        prev = finish_qpass(zq, *state[zq])
                        prev_qp = zq
                        prev_age = 0
                        projs_done = 0
                if qp == 0 and pair in (1, 2, 3, 4):
                    emit_qt_ch1(pair - 1)
                if qp == 0 and pair == 7:
                    emit_bp_matvec()
                prev_age += 1
                if (prev is not None and projs_done < CS and prev_age >= 4
                        and prev_age % 2 == 0):
                    emit_proj_cs(prev_qp, projs_done, *prev)
                    projs_done += 1
            while z_idx < len(all_pairs):
                zq, zp = all_pairs[z_idx]
                emit_z_pair(zq, zp, *state[zq])
                z_idx += 1
                if zp == NP2 - 1:
                    prev = finish_qpass(zq, *state[zq])
                    prev_qp = zq
                    projs_done = 0
            for cs in range(projs_done, CS):
                emit_proj_cs(prev_qp, cs, *prev, tail=True)

    nc.finalize()
    return nc


def make_consts(P=128, cpg=16):
    GPS = P // cpg
    indg = np.zeros((P, GPS), np.float32)
    for p in range(P):
        indg[p, p // cpg] = 1.0
    inde = indg.T.copy()
    return indg, inde


_PROGRAM_CACHE = {}


def _get_program(C, G, N, NQ, precision="tf32"):
    key = (C, G, N, NQ, precision)
    if key not in _PROGRAM_CACHE:
        _PROGRAM_CACHE[key] = build_program(C=C, G=G, N=N, NQ=NQ,
                                            precision=precision)
    return _PROGRAM_CACHE[key]


def make_in_maps(x, gn_w, gn_b, q_w, q_b, k_w, k_b, v_w, v_b, proj_w, proj_b,
                 n_cores=8, G=32):
    """Shard full inputs into per-core input maps (weight products folded on
    host).  Per-core x is pixel-permuted so the core's query block is first;
    attention is permutation-invariant over keys so S/Z stay consistent."""
    import ml_dtypes
    bf = ml_dtypes.bfloat16
    f8 = ml_dtypes.float8_e4m3
    f = lambda a: np.ascontiguousarray(np.asarray(a, dtype=np.float32))
    x = f(x)
    b, c, h, w = x.shape
    n = h * w
    qblocks = n_cores // b
    nq = n // qblocks
    cs = c // 128
    kt = n // 128
    gps = 128 // (c // G)
    scale = np.float32(c ** -0.5)
    xf = x.reshape(b, c, n)

    def to_pcs(v):                       # [C] -> [128, CS] (c = 128*s + p)
        return np.asarray(v, np.float32).reshape(cs, 128).T

    qw, kw, vw, pw = f(q_w), f(k_w), f(v_w), f(proj_w)
    indg, inde = make_consts(cpg=c // G)
    cpk = np.zeros((128, 4 * cs + gps + 128), np.float32)
    cpk[:, 0:cs] = to_pcs(f(gn_w))
    cpk[:, cs:2 * cs] = to_pcs(f(gn_b))
    cpk[:, 2 * cs:3 * cs] = to_pcs(64.0 * scale * (kw.T @ f(q_b)))
    cpk[:, 3 * cs:4 * cs] = to_pcs(pw @ f(v_b) + f(proj_b))
    cpk[:, 4 * cs:4 * cs + gps] = indg
    cpk[0:gps, 4 * cs + gps:] = inde
    common = {
        # pre-scaled by SC_P0=64: the device fold is then just the A
        # row-scale, and drains divide the 64 back out
        "p0t": np.ascontiguousarray((64.0 * scale * (qw.T @ kw)).astype(bf)),
        "w2t": np.ascontiguousarray((pw @ vw).T.astype(bf)),
        "cpk": cpk,
    }
    in_maps = []
    for i in range(n_cores):
        bi, qi = divmod(i, qblocks)
        xb = xf[bi]
        qs, qe = qi * nq, (qi + 1) * nq
        xperm = np.concatenate([xb[:, qs:qe], xb[:, :qs], xb[:, qe:]], axis=1)
        x8 = xperm.astype(f8)
        xt8 = np.ascontiguousarray(
            x8.T.reshape(kt // 2, 2, 128, c).transpose(2, 0, 1, 3))
        in_maps.append({
            **common,
            # bf16 x ships only where bf16 stats windows + residual read it;
            # the last quarter's stats come from x8 on device
            "x": np.ascontiguousarray(xperm[:, :n - 1024].astype(bf)),
            "x8": np.ascontiguousarray(x8),
            "xt8": xt8,
        })
    return in_maps, (b, c, h, w, n, nq, qblocks)


def kernel(x, gn_w, gn_b, q_w, q_b, k_w, k_b, v_w, v_b, proj_w, proj_b):
    from concourse.bass_utils import run_bass_kernel_spmd

    in_maps, (b, c, h, w, n, nq, qblocks) = make_in_maps(
        x, gn_w, gn_b, q_w, q_b, k_w, k_b, v_w, v_b, proj_w, proj_b
    )
    n_cores = 8
    nc = _get_program(C=c, G=32, N=n, NQ=nq)
    res = run_bass_kernel_spmd(nc, in_maps, list(range(n_cores))).results
    out = np.empty((b, c, n), np.float32)
    for i in range(n_cores):
        bi, qi = divmod(i, qblocks)
        out[bi, :, qi * nq:(qi + 1) * nq] = res[i]["out"]
    return out.reshape(b, c, h, w)


# revision 76
# speedup vs baseline: 1.7578x; 1.0063x over previous
"""AttnBlock (GroupNorm -> QKV 1x1 -> full NxN attention -> proj -> residual)
for Trainium2, SPMD over 8 NeuronCores.

Sharding: data-parallel over batch (2) x query-pixel blocks (4 of 1024 px).
Each core receives its batch image x [C, N] PERMUTED so that its own query
block occupies pixels [0, NQ); attention is permutation-invariant over keys.
No collectives.

v3 structure — K and V are never materialized.  Weight products fold on the
host; GroupNorm folds into tiny per-channel vectors on device:

  hn = A*x + B (per-channel).  With P0 = s*Wk^T@Wq and W2 = Wp@Wv (host):
    S[k,q]  = x[:,k]^T @ QtA[:,q]   (+ per-q consts that cancel in softmax)
    QtA     = diag(A) (P0^T diag(A) x_q + P0^T B + s Wk^T bq)
    out     = W2 (A . Z0) / den + (W2 B + Wp bv + bp) + x,   Z0 = x @ P^T

  Device tensors: x bf16 (stats + residual), x8 fp8 [c,n] (S lhsT + query
  rhs), xt8 fp8 [k,c] pair-interleaved (Z lhsT; host-transposed), p0t bf16,
  w2t bf16, one packed const vector.  GroupNorm A enters via a [C,C] lhsT
  row-scale (P0A8 fold), a drain scale on Qt, and a drain scale on Z0; all
  B / bias terms ride along as drain biases or the proj bias.

  Heavy matmuls (Qt production, S, Z0, softmax denominators) run fp8 e4m3
  DoubleRow (0.5 cyc/row); proj runs bf16.

  Scheduling notes (cost-model driven):
  - All input DMAs ride the SP queue in priority order (p0t, x, x8, xt8,
    w2t); the DMA engines are a single ~25us serial resource, so order is
    everything.  Stats windows pipeline with the x chunks as they land.
  - Per 512-px window: DVE does the sums (tensor_scalar+accum runs 4x on
    bf16) + one sumsq; ACT two sumsq (Square+accum); Pool one sumsq.
  - ACT loads the Sqrt table set first (it also holds Identity/Square for
    the stats window), switches to the Exp set once, then owns the exp
    stream; every other drain lives on DVE/Pool.
  - S->exp->Z0 pipeline: S pairs on PE feed ACT exp; Z0/den DoubleRow
    matmuls trail one pair behind; the previous qpass's proj (and the
    bp_dev matvec) slot into PE gaps of the exp-bound stream.
"""

from contextlib import ExitStack

import numpy as np

import concourse.bacc as bacc
import concourse.bass as bass
import concourse.mybir as mybir
import concourse.tile as tile

F32 = mybir.dt.float32
F32R = mybir.dt.float32r
BF16 = mybir.dt.bfloat16
FP8 = mybir.dt.float8e4
AF = mybir.ActivationFunctionType
MUL = mybir.AluOpType.mult
ADD = mybir.AluOpType.add

SC_P0 = 64.0      # fp8 P0A lhsT pre-scale (dodges e4m3 subnormals)
SC_QT = 16.0      # fp8 Qt storage scale


def build_program(C=512, G=32, N=4096, NQ=1024, eps=1e-5, precision="tf32"):
    """Emit the per-core Bass program (SPMD; per-core data differs only)."""
    P = 128
    CS = C // P                  # channel subtiles
    KT = N // P                  # key/pixel tiles
    NCH = 512                    # x DMA chunk / stats window (px)
    NCHUNKS = N // NCH
    QP = min(512, NQ)            # query-pass width
    QPASSES = NQ // QP
    cpg = C // G                 # channels per group
    GPS = P // cpg               # groups per channel-subtile
    assert C % P == 0 and N % P == 0 and NQ % QP == 0 and P % cpg == 0
    SDT = BF16

    nc = bacc.Bacc(None, target_bir_lowering=False)

    NX = N - 1024        # bf16 x ships only for stats windows + residual;
    x_d = nc.dram_tensor("x", [C, NX], SDT, kind="ExternalInput")
    x8_d = nc.dram_tensor("x8", [C, N], FP8, kind="ExternalInput")
    xt8_d = nc.dram_tensor("xt8", [P, KT // 2, 2, C], FP8, kind="ExternalInput")
    p0t_d = nc.dram_tensor("p0t", [C, C], SDT, kind="ExternalInput")
    w2t_d = nc.dram_tensor("w2t", [C, C], SDT, kind="ExternalInput")
    CPW = 4 * CS + GPS + P
    cpk_d = nc.dram_tensor("cpk", [P, CPW], F32, kind="ExternalInput")
    out_d = nc.dram_tensor("out", [C, NQ], SDT, kind="ExternalOutput")

    x_r = x_d[:, :].rearrange("(s p) n -> p s n", p=P)
    x8_r = x8_d[:, :].rearrange("(s p) n -> p s n", p=P)
    p0_r = p0t_d[:, :].rearrange("(s p) o -> p s o", p=P)
    w2_r = w2t_d[:, :].rearrange("(s p) o -> p s o", p=P)
    out_r = out_d[:, :].rearrange("(s p) n -> p s n", p=P)

    with tile.TileContext(nc) as tc, ExitStack() as st:
        const = st.enter_context(tc.tile_pool(name="const", bufs=1))
        big = st.enter_context(tc.tile_pool(name="big", bufs=1))
        small = st.enter_context(tc.tile_pool(name="small", bufs=1))
        ptp = st.enter_context(tc.tile_pool(name="ptp", bufs=2 * (KT // 2)))

        # resident big tensors
        x_sb = big.tile([P, CS, NX], SDT, tag="x")         # x bf16 (partial)
        x8 = big.tile([P, CS, N], FP8, tag="x8")           # x fp8 [c, n]
        xt8 = big.tile([P, KT // 2, 2, C], FP8, tag="xt8")  # x^T fp8 pairs
        p0t = big.tile([P, CS, C], SDT, tag="p0t")         # s*Wk^T Wq (lhsT)
        p0a8 = big.tile([P, CS, C], FP8, tag="p0a8")       # A-folded fp8 P0
        w2t = big.tile([P, CS, C], SDT, tag="w2t")         # Wp@Wv (lhsT)
        qt8 = big.tile([P, CS, NQ], FP8, tag="qt8")        # QtA fp8
        cpk = const.tile([P, CPW], F32, tag="cpk")

        # ---- input DMAs: one queue (SP), priority order --------------------
        # The last quarter's stats come from x8, so its chunk goes FIRST
        # (ACT starts those reductions at ~2us) and bf16 x ships only
        # [0, NX).  cpack feeds the stats combine at ~18us.
        nc.sync.dma_start(out=x8[:, :, 3 * 1024:4 * 1024],
                          in_=x8_r[:, :, 3 * 1024:4 * 1024])
        for qd in range(NX // NCH):
            nc.sync.dma_start(out=x_sb[:, :, qd * NCH:(qd + 1) * NCH],
                              in_=x_r[:, :, qd * NCH:(qd + 1) * NCH])
        nc.sync.dma_start(out=cpk, in_=cpk_d[:, :])
        nc.sync.dma_start(out=p0t, in_=p0_r)
        # x8 and xt8 interleaved per 1024-px chunk: the S and Z streams
        # consume pixels in the same order, so each operand pair lands just
        # ahead of its first use instead of Z head-blocking on a late xt8
        for qd in range(4):
            if qd < 3:
                nc.sync.dma_start(
                    out=x8[:, :, qd * 1024:(qd + 1) * 1024],
                    in_=x8_r[:, :, qd * 1024:(qd + 1) * 1024])
            nc.sync.dma_start(
                out=xt8[:, qd * (KT // 8):(qd + 1) * (KT // 8), :, :],
                in_=xt8_d[:, qd * (KT // 8):(qd + 1) * (KT // 8), :, :])
        nc.sync.dma_start(out=w2t, in_=w2_r)

        gammaT = cpk[:, 0:CS]
        betaT = cpk[:, CS:2 * CS]
        h0T = cpk[:, 2 * CS:3 * CS]
        bp2T = cpk[:, 3 * CS:4 * CS]
        indg = cpk[:, 4 * CS:4 * CS + GPS]
        inde = cpk[0:GPS, 4 * CS + GPS:4 * CS + GPS + P]

        with ExitStack() as st1:
            ps_sm = st1.enter_context(tc.tile_pool(name="ps_sm", bufs=2,
                                                   space="PSUM"))
            ps_qt = st1.enter_context(tc.tile_pool(name="ps_qt", bufs=4,
                                                   space="PSUM"))

            nc0_t = const.tile([P, 1], F32, tag="nc0")   # exp shift (fp8 rng)
            nc.vector.memset(nc0_t, -2.5)
            ones8 = const.tile([P, 2, P], FP8, tag="ones8")  # denom lhsT
            nc.vector.memset(ones8, 1.0)
            # single ACT table load for the whole kernel: the Exp set also
            # holds Identity/Square (stats + folds); rsqrt happens on DVE
            # via Newton, so Sqrt's set is never needed.  Loading now also
            # wins the DMA-engine queue before the big input transfers.
            dume = small.tile([P, 1], F32, tag="dume")
            nc.scalar.activation(out=dume, in_=nc0_t, func=AF.Exp)

            # ---- phase 1: GroupNorm stats, pipelined with the x DMAs ------
            # Pool cannot reduce (no accum) and tensor_tensor_reduce does
            # not exist on hw.  The LAST quarter's stats come from the
            # early-DMA'd x8 chunk, entirely on ACT (Identity-accum sums +
            # Square-accum sumsq, running from ~2us while the bf16 x
            # streams).  bf16 px: DVE bn_stats on 4 windows + one double
            # window as ACT Square / DVE 4x-mode tensor_scalar sums.
            WIN_DVE = [0, 1, 2, 3]
            BF_SUM = (2048, 3072)     # bf16 dbl window 4+5
            nA = len(WIN_DVE) * NCH
            stats_all = small.tile([P, CS, len(WIN_DVE), 6], F32, tag="stats")
            sxa = small.tile([P, CS, 2, 2], F32, tag="sxa")
            scr = small.tile([P, 2, 1024], SDT, tag="scr")
            # fp8 quarter: its x8 chunk lands at ~1.6us while bf16 x is
            # still streaming, so the sum pass splits DVE (idle until the
            # first bf16 chunk) / ACT to balance the two stats queues
            for s in range(2):
                nc.vector.tensor_scalar(
                    out=scr[:, 1, :], in0=x8[:, s, 3072:4096],
                    scalar1=1.0, scalar2=0.0, op0=MUL, op1=ADD,
                    accum_out=sxa[:, s, 1, 0:1])
            for s in range(2, CS):
                nc.scalar.activation(out=scr[:, 0, :],
                                     in_=x8[:, s, 3072:4096],
                                     func=AF.Identity,
                                     accum_out=sxa[:, s, 1, 0:1])
            for s in range(CS):
                nc.scalar.activation(out=scr[:, 0, :],
                                     in_=x8[:, s, 3072:4096],
                                     func=AF.Square,
                                     accum_out=sxa[:, s, 1, 1:2])
            for wi, w0 in enumerate(WIN_DVE):
                for s in range(CS):
                    nc.vector.bn_stats(
                        out=stats_all[:, s, wi, :],
                        in_=x_sb[:, s, w0 * NCH:(w0 + 1) * NCH])
            for s in range(CS):
                nc.vector.tensor_scalar(
                    out=scr[:, 1, :], in0=x_sb[:, s, BF_SUM[0]:BF_SUM[1]],
                    scalar1=1.0, scalar2=0.0, op0=MUL, op1=ADD,
                    accum_out=sxa[:, s, 0, 0:1])
            for s in range(CS):
                nc.scalar.activation(out=scr[:, 0, :],
                                     in_=x_sb[:, s, BF_SUM[0]:BF_SUM[1]],
                                     func=AF.Square,
                                     accum_out=sxa[:, s, 0, 1:2])
            mv = small.tile([P, CS, 2], F32, tag="mv")
            for s in range(CS):
                nc.vector.bn_aggr(out=mv[:, s, :], in_=stats_all[:, s, :, :])

            # combine: the group reduction accumulates the three stats
            # sources directly in PSUM as each lands (no serial add-chain):
            # ps_g[g] = sum over sources of indg^T @ [sums | sumsqs]
            rhs8 = small.tile([P, 2 * CS], F32, tag="rhs8")
            nc.vector.tensor_scalar_mul(rhs8[:, 0:CS], mv[:, :, 0], float(nA))
            nc.vector.tensor_mul(out=rhs8[:, CS:], in0=mv[:, :, 0],
                                 in1=mv[:, :, 0])
            nc.vector.tensor_add(out=rhs8[:, CS:], in0=rhs8[:, CS:],
                                 in1=mv[:, :, 1])
            nc.vector.tensor_scalar_mul(rhs8[:, CS:], rhs8[:, CS:], float(nA))
            ps_g = ps_sm.tile([GPS, 2 * CS], F32, tag="sm", name="ps_g")
            srcs = [sxa[:, :, 1, 0], rhs8[:, 0:CS], sxa[:, :, 0, 0]]
            sqs = [sxa[:, :, 1, 1], rhs8[:, CS:], sxa[:, :, 0, 1]]
            for i in range(3):   # ordered by expected readiness
                nc.tensor.matmul(ps_g[:, 0:CS], lhsT=indg, rhs=srcs[i],
                                 start=(i == 0), stop=(i == 2),
                                 skip_group_check=True)
                nc.tensor.matmul(ps_g[:, CS:], lhsT=indg, rhs=sqs[i],
                                 start=(i == 0), stop=(i == 2),
                                 skip_group_check=True)
            gtmp = small.tile([GPS, 2 * CS], F32, tag="gtmp")
            nc.vector.tensor_scalar_mul(gtmp, ps_g, 1.0 / (cpg * N))
            # gvar = E[x^2] - mean^2 ; grstd = 1/sqrt(gvar + eps)
            gsq = small.tile([GPS, CS], F32, tag="gsq")
            nc.vector.tensor_mul(out=gsq, in0=gtmp[:, 0:CS], in1=gtmp[:, 0:CS])
            e8 = small.tile([GPS, 2 * CS], F32, tag="e8")
            wv = small.tile([GPS, CS], F32, tag="wv")
            nc.vector.scalar_tensor_tensor(   # w = (E[x^2]+eps) - mean^2
                out=wv, in0=gtmp[:, CS:], scalar=eps, in1=gsq,
                op0=ADD, op1=mybir.AluOpType.subtract)
            # rstd = rsqrt(w) by Newton on DVE (w ~ 1 for normalized input;
            # seed 1.5 - w/2 is the tangent at 1, two steps to fp32 noise)
            y_t = e8[:, 0:CS]
            nc.vector.tensor_scalar(out=y_t, in0=wv, scalar1=-0.5,
                                    scalar2=1.5, op0=MUL, op1=ADD)
            nwt = small.tile([GPS, CS], F32, tag="nwt")
            for _ in range(1):   # seed err ~4e-3 -> ~2e-5 after one step
                nc.vector.tensor_mul(out=nwt, in0=y_t, in1=y_t)
                nc.vector.tensor_mul(out=nwt, in0=nwt, in1=wv)
                nc.vector.tensor_scalar(out=nwt, in0=nwt, scalar1=-0.5,
                                        scalar2=1.5, op0=MUL, op1=ADD)
                nc.vector.tensor_mul(out=y_t, in0=y_t, in1=nwt)
            nc.vector.tensor_copy(out=e8[:, CS:], in_=gtmp[:, 0:CS])
            # expand groups -> channels
            ps_e = ps_sm.tile([P, 2 * CS], F32, tag="sm", name="ps_e")
            nc.tensor.matmul(ps_e, lhsT=inde, rhs=e8, start=True, stop=True)
            A_sb = small.tile([P, CS], F32, tag="A")     # A = gamma * rstd
            nc.vector.tensor_mul(out=A_sb, in0=ps_e[:, 0:CS], in1=gammaT)
            B32 = small.tile([P, CS], F32, tag="B32")    # B = beta - A*mean
            nc.vector.tensor_mul(out=B32, in0=ps_e[:, CS:], in1=A_sb)
            nc.vector.tensor_sub(out=B32, in0=betaT, in1=B32)
            B_sb = small.tile([P, CS], SDT, tag="B")
            nc.vector.tensor_copy(out=B_sb, in_=B32)

            # ---- phase 2: P0A fold, bias matvec, Qt production ------------
            # p0t ships pre-scaled by SC_P0 from the host, so the fold is
            # just the A row-scale; all 4 subtiles gate every Qt matmul, so
            # split it DVE/ACT (Identity is in the Exp set)
            # three-engine fold (Pool's AP tensor_scalar_mul is the proven
            # baseline wv8-fold form): wall ~0.9us instead of two rounds
            FOLD_ENG = {0: nc.vector, 3: nc.vector, 1: nc.gpsimd}
            for s in range(CS):
                if s == 2:
                    nc.scalar.activation(
                        out=p0a8[:, s, :], in_=p0t[:, s, :],
                        func=AF.Identity, scale=A_sb[:, s:s + 1])
                else:
                    FOLD_ENG[s].tensor_scalar_mul(
                        p0a8[:, s, :], p0t[:, s, :], A_sb[:, s:s + 1])

            # r0 = P0^T B + h0 (Q-bias term of S, varies per key channel)
            ps_r = ps_sm.tile([P, CS], F32, tag="sm", name="ps_r")
            for cs in range(CS):
                for s in range(CS):
                    nc.tensor.matmul(
                        ps_r[:, cs:cs + 1],
                        lhsT=p0t[:, s, cs * P:(cs + 1) * P],
                        rhs=B_sb[:, s:s + 1],
                        start=(s == 0), stop=(s == CS - 1),
                        skip_group_check=True,
                    )
            # ps_r and h0T carry the host-side SC_P0 factor; the drain
            # scale/bias divide it back out
            qdr_s = small.tile([P, CS], F32, tag="qdr_s")
            nc.vector.tensor_scalar_mul(qdr_s, A_sb, SC_QT / SC_P0)
            # per-cs: the matvec is cs-major, so column 0's bias is ready
            # after 4 matmuls and the first Qt drain needn't wait for all 16
            qdr_b = small.tile([P, CS], F32, tag="qdr_b")
            for c_ in range(CS):
                nc.vector.tensor_add(out=qdr_b[:, c_:c_ + 1],
                                     in0=ps_r[:, c_:c_ + 1],
                                     in1=h0T[:, c_:c_ + 1])
                nc.vector.scalar_tensor_tensor(
                    out=qdr_b[:, c_:c_ + 1], in0=qdr_b[:, c_:c_ + 1],
                    scalar=SC_QT / SC_P0, in1=A_sb[:, c_:c_ + 1],
                    op0=MUL, op1=MUL)

            # Qt production: QtA8 = fp8(SC_QT*A_o*(psum/SC_P0 + r0_o)).
            # Only ch0 (qp0's query columns) gates the stream; it drains
            # split DVE/ACT (Identity with scale+bias APs is hw-verified).
            # ch1 is deferred into the early exp stream as PE filler.
            for cs in range(CS):
                ps_q = ps_qt.tile([P, 512], F32, tag="qt")
                for t in range(CS // 2):
                    nc.tensor.matmul(
                        ps_q,
                        lhsT=p0a8[:, 2 * t:2 * t + 2, cs * P:(cs + 1) * P],
                        rhs=x8[:, 2 * t:2 * t + 2, 0:512],
                        start=(t == 0), stop=(t == CS // 2 - 1),
                        perf_mode=mybir.MatmulPerfMode.DoubleRow,
                    )
                if cs % 2 == 0:
                    nc.vector.tensor_scalar(
                        out=qt8[:, cs, 0:512], in0=ps_q,
                        scalar1=qdr_s[:, cs:cs + 1],
                        scalar2=qdr_b[:, cs:cs + 1],
                        op0=MUL, op1=ADD,
                    )
                else:
                    nc.scalar.activation(
                        out=qt8[:, cs, 0:512], in_=ps_q, func=AF.Identity,
                        scale=qdr_s[:, cs:cs + 1],
                        bias=qdr_b[:, cs:cs + 1],
                    )

        # ---- phase 3: S -> exp -> Z0/den stream + proj + residual ---------
        with ExitStack() as st2:
            ocq = st2.enter_context(tc.tile_pool(name="ocq", bufs=2))
            outp = st2.enter_context(tc.tile_pool(name="outp", bufs=2))
            sm2 = st2.enter_context(tc.tile_pool(name="sm2", bufs=2))
            # 3 banks S stream (shared with proj psum) + 5 banks Z0/den
            ps_s = st2.enter_context(tc.tile_pool(name="ps_s", bufs=3,
                                                  space="PSUM"))
            ps_o = st2.enter_context(tc.tile_pool(name="ps_o", bufs=CS + 1,
                                                  space="PSUM"))

            bp_dev = small.tile([P, CS], F32, tag="bp")
            pt_tiles = {}

            def emit_qt_ch1(cs):
                # deferred Qt columns for qp1 (needed at g=16): fills the
                # early-stream PE gaps; drains stay off ACT's exp queue
                ps_q = ps_s.tile([P, 512], F32, tag="sbank",
                                 name=f"qt1_{cs}")
                for t in range(CS // 2):
                    nc.tensor.matmul(
                        ps_q,
                        lhsT=p0a8[:, 2 * t:2 * t + 2, cs * P:(cs + 1) * P],
                        rhs=x8[:, 2 * t:2 * t + 2, 512:1024],
                        start=(t == 0), stop=(t == CS // 2 - 1),
                        perf_mode=mybir.MatmulPerfMode.DoubleRow,
                    )
                nc.vector.tensor_scalar(
                    out=qt8[:, cs, 512:1024], in0=ps_q,
                    scalar1=qdr_s[:, cs:cs + 1],
                    scalar2=qdr_b[:, cs:cs + 1],
                    op0=MUL, op1=ADD,
                )

            def emit_s_pair(qp_, pair):
                q0_ = qp_ * QP
                pt = ptp.tile([P, 2, QP], FP8, tag="pt",
                              name=f"pt_{qp_}_{pair}")
                pt_tiles[(qp_, pair)] = pt
                for half in range(2):
                    kt = 2 * pair + half
                    s_ps = ps_s.tile([P, QP], F32, tag="sbank",
                                     name=f"s_ps_{qp_}_{kt}")
                    for t in range(CS // 2):
                        nc.tensor.matmul(
                            s_ps,
                            lhsT=x8[:, 2 * t:2 * t + 2, kt * P:(kt + 1) * P],
                            rhs=qt8[:, 2 * t:2 * t + 2, q0_:q0_ + QP],
                            start=(t == 0), stop=(t == CS // 2 - 1),
                            perf_mode=mybir.MatmulPerfMode.DoubleRow,
                        )
                    nc.scalar.activation(out=pt[:, half, :], in_=s_ps,
                                         func=AF.Exp, bias=nc0_t,
                                         scale=1.0 / SC_QT)

            def emit_z_pair(qp_, pair, o_ps, den_ps):
                pt = pt_tiles[(qp_, pair)]
                last = pair == KT // 2 - 1
                for cs in range(CS):
                    nc.tensor.matmul(
                        o_ps[cs],
                        lhsT=xt8[:, pair, :, cs * P:(cs + 1) * P],
                        rhs=pt,
                        start=(pair == 0), stop=last,
                        perf_mode=mybir.MatmulPerfMode.DoubleRow,
                    )
                nc.tensor.matmul(
                    den_ps, lhsT=ones8, rhs=pt,
                    start=(pair == 0), stop=last,
                    perf_mode=mybir.MatmulPerfMode.DoubleRow,
                )

            def emit_bp_matvec():
                # bp_dev = W2 @ B + (Wp bv + bp); w2t lands late, so this
                # slots into the qp0 stream well after the fold
                ps_z = ps_s.tile([P, CS], F32, tag="sbank", name="ps_z")
                for cs in range(CS):
                    for s in range(CS):
                        nc.tensor.matmul(
                            ps_z[:, cs:cs + 1],
                            lhsT=w2t[:, s, cs * P:(cs + 1) * P],
                            rhs=B_sb[:, s:s + 1],
                            start=(s == 0), stop=(s == CS - 1),
                            skip_group_check=True,
                        )
                nc.vector.tensor_add(out=bp_dev, in0=ps_z, in1=bp2T)

            def emit_proj_cs(qp_, cs, oc, rec_bc, ot, tt, tail=False):
                q0_ = qp_ * QP
                ps_pp = ps_s.tile([P, QP], F32, tag="sbank",
                                  name=f"pp_{qp_}_{cs}")
                for s in range(CS):
                    nc.tensor.matmul(
                        ps_pp, lhsT=w2t[:, s, cs * P:(cs + 1) * P],
                        rhs=oc[:, s, :],
                        start=(s == 0), stop=(s == CS - 1),
                    )
                # tt reads PSUM -> DVE.  Mid-stream the bias+residual adds
                # ride idle Pool (scalar_tensor_tensor is illegal there, so
                # two ops); on the latency-critical tail they spread across
                # DVE (1-op stt) and ACT(+bias)/Pool(+x).
                nc.vector.tensor_mul(out=tt[:, cs, :], in0=ps_pp, in1=rec_bc)
                if tail and cs % 2 == 1:
                    nc.vector.scalar_tensor_tensor(
                        out=ot[:, cs, :], in0=tt[:, cs, :],
                        scalar=bp_dev[:, cs:cs + 1],
                        in1=x_sb[:, cs, q0_:q0_ + QP],
                        op0=ADD, op1=ADD)
                else:
                    if tail:
                        nc.scalar.activation(out=ot[:, cs, :],
                                             in_=tt[:, cs, :],
                                             func=AF.Identity,
                                             bias=bp_dev[:, cs:cs + 1])
                    else:
                        nc.gpsimd.tensor_scalar_add(ot[:, cs, :],
                                                    tt[:, cs, :],
                                                    bp_dev[:, cs:cs + 1])
                    nc.gpsimd.tensor_add(out=ot[:, cs, :], in0=ot[:, cs, :],
                                         in1=x_sb[:, cs, q0_:q0_ + QP])
                # out rides SP while streaming (a waiting dma_start holds its
                # engine's SEQ); on the tail ACT is free and shares the load
                eng = nc.scalar if (tail and cs % 2 == 1) else nc.sync
                eng.dma_start(
                    out=out_r[:, cs, q0_:q0_ + QP], in_=ot[:, cs, :])

            def finish_qpass(qp, o_ps, den_ps):
                # Z0 drains first (they gate the proj matmuls); the den
                # reciprocal is only needed ~3us later by tt
                oc = ocq.tile([P, CS, QP], SDT, tag="ocq")
                for cs in range(CS):
                    # mid-stream qpasses keep ACT free for exp; the final
                    # qpass's drains split DVE/ACT (exp stream is over)
                    if qp < QPASSES - 1 or cs % 2 == 0:
                        nc.vector.tensor_scalar_mul(oc[:, cs, :], o_ps[cs],
                                                    A_sb[:, cs:cs + 1])
                    else:
                        nc.scalar.activation(out=oc[:, cs, :], in_=o_ps[cs],
                                             func=AF.Identity,
                                             scale=A_sb[:, cs:cs + 1])
                rec_bc = sm2.tile([P, QP], F32, tag="recbc", name=f"rb_{qp}")
                nc.vector.reciprocal(out=rec_bc, in_=den_ps)
                ot = outp.tile([P, CS, QP], SDT, tag="ot")
                tt = outp.tile([P, CS, QP], F32, tag="tt")
                return (oc, rec_bc, ot, tt)

            # One global Z stream trailing the S stream by ZLAG pairs: it
            # rides out the late xt8 DMA arrival, keeps PE fed while ACT
            # exps, and crosses qpass boundaries without stalling (qp0's Z
            # tail drains inside qp1's S stream).  The previous qpass's
            # proj chains then slot into PE gaps ~4 pairs later, once its
            # Z0 drains have cleared DVE.
            NP2 = KT // 2
            all_pairs = [(qp, pair) for qp in range(QPASSES)
                         for pair in range(NP2)]
            state = {}
            z_idx = 0
            prev = None
            prev_qp = -1
            prev_age = 0
            projs_done = CS
            for g, (qp, pair) in enumerate(all_pairs):
                if pair == 0:
                    o_ps = [ps_o.tile([P, QP], F32, tag="o",
                                      name=f"o_{qp}_{c}") for c in range(CS)]
                    den_ps = ps_o.tile([P, QP], F32, tag="o",
                                       name=f"den_{qp}")
                    state[qp] = (o_ps, den_ps)
                emit_s_pair(qp, pair)
                zlag = 1
                budget = 2
                while z_idx <= g - zlag and budget > 0:
                    zq, zp = all_pairs[z_idx]
                    emit_z_pair(zq, zp, *state[zq])
                    z_idx += 1
                    budget -= 1
                    if zp == NP2 - 1:
                        prev = finish_qpass(zq, *state[zq])
                        prev_qp = zq
                        prev_age = 0
                        projs_done = 0
                if qp == 0 and pair in (1, 2, 3, 4):
                    emit_qt_ch1(pair - 1)
                if qp == 0 and pair == 7:
                    emit_bp_matvec()
                prev_age += 1
                if (prev is not None and projs_done < CS and prev_age >= 4
                        and prev_age % 2 == 0):
                    emit_proj_cs(prev_qp, projs_done, *prev)
                    projs_done += 1
            while z_idx < len(all_pairs):
                zq, zp = all_pairs[z_idx]
                emit_z_pair(zq, zp, *state[zq])
                z_idx += 1
                if zp == NP2 - 1:
                    prev = finish_qpass(zq, *state[zq])
                    prev_qp = zq
                    projs_done = 0
            for cs in range(projs_done, CS):
                emit_proj_cs(prev_qp, cs, *prev, tail=True)

    nc.finalize()
    return nc


def make_consts(P=128, cpg=16):
    GPS = P // cpg
    indg = np.zeros((P, GPS), np.float32)
    for p in range(P):
        indg[p, p // cpg] = 1.0
    inde = indg.T.copy()
    return indg, inde


_PROGRAM_CACHE = {}


def _get_program(C, G, N, NQ, precision="tf32"):
    key = (C, G, N, NQ, precision)
    if key not in _PROGRAM_CACHE:
        _PROGRAM_CACHE[key] = build_program(C=C, G=G, N=N, NQ=NQ,
                                            precision=precision)
    return _PROGRAM_CACHE[key]


def make_in_maps(x, gn_w, gn_b, q_w, q_b, k_w, k_b, v_w, v_b, proj_w, proj_b,
                 n_cores=8, G=32):
    """Shard full inputs into per-core input maps (weight products folded on
    host).  Per-core x is pixel-permuted so the core's query block is first;
    attention is permutation-invariant over keys so S/Z stay consistent."""
    import ml_dtypes
    bf = ml_dtypes.bfloat16
    f8 = ml_dtypes.float8_e4m3
    f = lambda a: np.ascontiguousarray(np.asarray(a, dtype=np.float32))
    x = f(x)
    b, c, h, w = x.shape
    n = h * w
    qblocks = n_cores // b
    nq = n // qblocks
    cs = c // 128
    kt = n // 128
    gps = 128 // (c // G)
    scale = np.float32(c ** -0.5)
    xf = x.reshape(b, c, n)

    def to_pcs(v):                       # [C] -> [128, CS] (c = 128*s + p)
        return np.asarray(v, np.float32).reshape(cs, 128).T

    qw, kw, vw, pw = f(q_w), f(k_w), f(v_w), f(proj_w)
    indg, inde = make_consts(cpg=c // G)
    cpk = np.zeros((128, 4 * cs + gps + 128), np.float32)
    cpk[:, 0:cs] = to_pcs(f(gn_w))
    cpk[:, cs:2 * cs] = to_pcs(f(gn_b))
    cpk[:, 2 * cs:3 * cs] = to_pcs(64.0 * scale * (kw.T @ f(q_b)))
    cpk[:, 3 * cs:4 * cs] = to_pcs(pw @ f(v_b) + f(proj_b))
    cpk[:, 4 * cs:4 * cs + gps] = indg
    cpk[0:gps, 4 * cs + gps:] = inde
    common = {
        # pre-scaled by SC_P0=64: the device fold is then just the A
        # row-scale, and drains divide the 64 back out
        "p0t": np.ascontiguousarray((64.0 * scale * (qw.T @ kw)).astype(bf)),
        "w2t": np.ascontiguousarray((pw @ vw).T.astype(bf)),
        "cpk": cpk,
    }
    in_maps = []
    for i in range(n_cores):
        bi, qi = divmod(i, qblocks)
        xb = xf[bi]
        qs, qe = qi * nq, (qi + 1) * nq
        xperm = np.concatenate([xb[:, qs:qe], xb[:, :qs], xb[:, qe:]], axis=1)
        x8 = xperm.astype(f8)
        xt8 = np.ascontiguousarray(
            x8.T.reshape(kt // 2, 2, 128, c).transpose(2, 0, 1, 3))
        in_maps.append({
            **common,
            # bf16 x ships only where bf16 stats windows + residual read it;
            # the last quarter's stats come from x8 on device
            "x": np.ascontiguousarray(xperm[:, :n - 1024].astype(bf)),
            "x8": np.ascontiguousarray(x8),
            "xt8": xt8,
        })
    return in_maps, (b, c, h, w, n, nq, qblocks)


def kernel(x, gn_w, gn_b, q_w, q_b, k_w, k_b, v_w, v_b, proj_w, proj_b):
    from concourse.bass_utils import run_bass_kernel_spmd

    in_maps, (b, c, h, w, n, nq, qblocks) = make_in_maps(
        x, gn_w, gn_b, q_w, q_b, k_w, k_b, v_w, v_b, proj_w, proj_b
    )
    n_cores = 8
    nc = _get_program(C=c, G=32, N=n, NQ=nq)
    res = run_bass_kernel_spmd(nc, in_maps, list(range(n_cores))).results
    out = np.empty((b, c, n), np.float32)
    for i in range(n_cores):
        bi, qi = divmod(i, qblocks)
        out[bi, :, qi * nq:(qi + 1) * nq] = res[i]["out"]
    return out.reshape(b, c, h, w)
